# revision 1
# baseline (speedup 1.0000x reference)
"""GAT (2-layer graph attention + output projection) on 8 Trainium2 NeuronCores.

Sharding: edges partitioned by destination node (1D graph partition, dst-sorted);
each core owns N/8 destination nodes and all edges pointing into them.
Per layer: each core computes ft = x @ [W | wr] for its node shard (er per node as
extra matmul columns), the ft tables are AllGathered, then each core gathers
ft[src] rows for its edges with dma_gather (two passes over the node space to fit
int16 indices), builds per-edge attention weights, and scatter-reduces messages
into per-node-chunk PSUM accumulators via one-hot matmuls. Segment softmax is
computed without max subtraction (logits are O(1) here, exp is safe) so only
segment sums are needed; normalization happens in the chunk epilogue.
"""
import math
import numpy as np

import concourse.bass as bass
import concourse.tile as tile
from concourse import bacc, mybir
from concourse.bass_utils import run_bass_kernel_spmd
from concourse.masks import make_identity
from contextlib import ExitStack

F32 = mybir.dt.float32
BF16 = mybir.dt.bfloat16
I16 = mybir.dt.int16

NCORE = 8
H, Dh = 4, 32
NEG = 0.1
HALF = 32768  # int16-addressable row limit for dma_gather
P = 128


# ----------------------------------------------------------------------------
# host-side graph preprocessing
# ----------------------------------------------------------------------------
def _edge_plan(src, dst, w, n_nodes, nloc, nlp, G):
    """Build per-core edge-slot arrays.

    Returns dict with T_LO, T_HI, group sizes, and per-core arrays:
      srclo_w [128, *], srchi_w [128, *] (wrapped int16 gather indices),
      erix_w [128, *], dstrel [128, S_tot] f32, wcol [128, S_tot] f32.
    """
    nch = nlp // P
    half = min(HALF, NCORE * nlp)
    core_of = dst // nloc
    loc = dst - core_of * nloc
    chunk = loc // P
    srcpad = (src // nloc) * nlp + (src % nloc)

    groups = [G] * (nch // G) + ([nch % G] if nch % G else [])

    # per (core, chunk) lo/hi edge lists
    per_cc_lo = [[[] for _ in range(nch)] for _ in range(NCORE)]
    per_cc_hi = [[[] for _ in range(nch)] for _ in range(NCORE)]
    order = np.argsort(core_of * (nch + 1) + chunk, kind="stable")
    for e in order:
        c, ch = int(core_of[e]), int(chunk[e])
        (per_cc_lo if srcpad[e] < half else per_cc_hi)[c][ch].append(int(e))

    T_LO = max(1, max(max(math.ceil(len(l) / P) for l in per_cc_lo[c]) for c in range(NCORE)))
    T_HI = max(1, max(max(math.ceil(len(l) / P) for l in per_cc_hi[c]) for c in range(NCORE)))
    T = T_LO + T_HI

    S_tot = sum(g * T for g in groups)
    LO_tot = sum(g * T_LO for g in groups) * P
    HI_tot = sum(g * T_HI for g in groups) * P

    def wrap(flat):
        # dma_gather idx layout: idx i -> partition i%16, col i//16; replicated x8
        a = flat.reshape(-1, 16).T.copy()
        return np.tile(a, (8, 1)).astype(np.int16)

    out = []
    for c in range(NCORE):
        srclo = np.zeros(LO_tot, np.int32)
        srchi = np.zeros(HI_tot, np.int32)
        erix = np.full(S_tot * P, nlp, np.int32)  # nlp = zero pad row of er table
        dstrel = np.full((P, S_tot), 1000.0, np.float32)
        wcol = np.zeros((P, S_tot), np.float32)

        s_off = 0      # slot offset (this group's first slot)
        lo_off = 0     # flat idx offset into srclo
        hi_off = 0
        ch0 = 0
        for g in groups:
            for cg in range(g):
                ch = ch0 + cg
                for region, lists, Treg, roff, arr, rbase in (
                    ("lo", per_cc_lo, T_LO, lo_off, srclo, 0),
                    ("hi", per_cc_hi, T_HI, hi_off, srchi, half),
                ):
                    edges = lists[c][ch]
                    for j, e in enumerate(edges):
                        t, p = j // P, j % P
                        i = (cg * Treg + t) * P + p
                        arr[roff + i] = srcpad[e] - rbase
                        if region == "lo":
                            s = s_off + cg * T_LO + t
                        else:
                            s = s_off + g * T_LO + cg * T_HI + t
                        erix[s * P + p] = loc[e]
                        dstrel[p, s] = float(loc[e] - ch * P)
                        wcol[p, s] = w[e]
            s_off += g * T
            lo_off += g * T_LO * P
            hi_off += g * T_HI * P
            ch0 += g
        out.append(dict(
            srclo_w=wrap(srclo), srchi_w=wrap(srchi), erix_w=wrap(erix),
            dstrel=dstrel, wcol=wcol,
        ))
    return dict(T_LO=T_LO, T_HI=T_HI, groups=groups, cores=out,
                S_tot=S_tot, LO_tot=LO_tot, HI_tot=HI_tot)


# ----------------------------------------------------------------------------
# device program
# ----------------------------------------------------------------------------
def _build_program(n_nodes, nloc, nlp, kin1, plan):
    """Build the 8-core SPMD Bass program. kin1 = layer-1 input feature dim."""
    T_LO, T_HI, groups = plan["T_LO"], plan["T_HI"], plan["groups"]
    T = T_LO + T_HI
    nch = nlp // P
    npad = NCORE * nlp
    hi_base = HALF if HALF < npad else 0
    ER_W = 64  # er table row width (floats) - 256B rows for dma_gather

    nc = bacc.Bacc("TRN2", target_bir_lowering=False, debug=False, num_devices=NCORE)

    # ---- I/O ----
    x0 = nc.dram_tensor("x0", [nlp, kin1], F32, kind="ExternalInput")
    w1e = nc.dram_tensor("w1e", [kin1, 132], F32, kind="ExternalInput")
    w2e = nc.dram_tensor("w2e", [128, 132], F32, kind="ExternalInput")
    wout = nc.dram_tensor("wout", [256, 128], F32, kind="ExternalInput")
    boutr = nc.dram_tensor("boutr", [1, 128], F32, kind="ExternalInput")
    al1r = nc.dram_tensor("al1r", [P, T * P], BF16, kind="ExternalInput")
    al2r = nc.dram_tensor("al2r", [P, T * P], BF16, kind="ExternalInput")
    iota = nc.dram_tensor("iota", [P, P], F32, kind="ExternalInput")
    onesr = nc.dram_tensor("onesr", [1, 128], F32, kind="ExternalInput")
    srclo = nc.dram_tensor("srclo", [P, plan["LO_tot"] // 16], I16, kind="ExternalInput")
    srchi = nc.dram_tensor("srchi", [P, plan["HI_tot"] // 16], I16, kind="ExternalInput")
    erix = nc.dram_tensor("erix", [P, plan["S_tot"] * P // 16], I16, kind="ExternalInput")
    dstrel = nc.dram_tensor("dstrel", [P, plan["S_tot"]], F32, kind="ExternalInput")
    wcol = nc.dram_tensor("wcol", [P, plan["S_tot"]], F32, kind="ExternalInput")
    out_local = nc.dram_tensor("out_local", [nlp, 128], F32, kind="ExternalOutput")

    # ---- internal DRAM ----
    ftl = [nc.dram_tensor(f"ft{l}_local", [nlp, 128], BF16) for l in (1, 2)]
    ftf = [nc.dram_tensor(f"ft{l}_full", [npad, 128], BF16, addr_space="Shared")
           for l in (1, 2)]
    ertab = [nc.dram_tensor(f"er{l}_tab", [nlp + P, ER_W], F32) for l in (1, 2)]
    x1l = nc.dram_tensor("x1_local", [nlp, 128], F32)
    x2l = nc.dram_tensor("x2_local", [nlp, 128], F32)

    rg = [list(range(NCORE))]

    with tile.TileContext(nc) as tc, ExitStack() as ctx:
        consts = ctx.enter_context(tc.tile_pool(name="consts", bufs=1))
        import os as _os0
        sb = ctx.enter_context(tc.tile_pool(name="sb", bufs=int(_os0.environ.get("KSBUFS", "2"))))
        sb_ft = ctx.enter_context(tc.tile_pool(name="sb_ft", bufs=3))
        ps_ft = ctx.enter_context(tc.tile_pool(name="ps_ft", bufs=2, space="PSUM"))  # tags: psf(2) + xt_ps(2) = 4 banks
        ps_e = ctx.enter_context(tc.tile_pool(name="ps_e", bufs=4, space="PSUM"))

        # ---- constants in SBUF ----
        iota_t = consts.tile([P, P], F32)
        nc.sync.dma_start(iota_t[:], iota[:, :])
        ones_t = consts.tile([1, 128], F32)
        nc.sync.dma_start(ones_t[:], onesr[:, :])
        bout_t = consts.tile([1, 128], F32)
        nc.sync.dma_start(bout_t[:], boutr[:, :])
        w1e_t = [consts.tile([P, 132], F32, name=f"w1e{k}", tag=f"w1e{k}") for k in range(kin1 // P)]
        for k in range(kin1 // P):
            nc.sync.dma_start(w1e_t[k][:], w1e[k * P:(k + 1) * P, :])
        w2e_t = consts.tile([P, 132], F32)
        nc.sync.dma_start(w2e_t[:], w2e[:, :])
        wout_t = [consts.tile([P, 128], F32, name=f"wout{k}", tag=f"wout{k}") for k in range(2)]
        for k in range(2):
            nc.sync.dma_start(wout_t[k][:], wout[k * P:(k + 1) * P, :])
        al_t = []
        for l, src_al in ((0, al1r), (1, al2r)):
            a = consts.tile([P, T * P], BF16, name=f"al{l}", tag=f"al{l}")
            nc.sync.dma_start(a[:], src_al[:, :])
            al_t.append(a)
        zer_t = consts.tile([1, ER_W], F32)
        nc.vector.memset(zer_t[:], 0.0)
        ident_t = consts.tile([P, P], F32)
        make_identity(nc, ident_t[:])

        def ft_phase(layer, x_src, kin, we_tiles, ftl_d, ertab_d, xT_src=None):
            er_stage = sb_ft.tile([P, nch, ER_W], F32, tag="er_stage")
            nc.vector.memset(er_stage[:], 0.0)
            for c in range(nch):
                if xT_src is None:
                    xc = sb_ft.tile([P, kin], F32, tag="xc")
                    nc.sync.dma_start(xc[:], x_src[c * P:(c + 1) * P, :])
                psf = ps_ft.tile([P, 132], F32, space="PSUM", tag="psf")
                for k in range(kin // P):
                    xt = sb_ft.tile([P, P], F32, tag="xt")
                    if xT_src is None:
                        xt_ps = ps_ft.tile([P, P], F32, space="PSUM", tag="xt_ps")
                        nc.tensor.transpose(xt_ps[:], xc[:, k * P:(k + 1) * P], ident_t[:])
                        nc.vector.tensor_copy(xt[:], xt_ps[:])
                    else:
                        nc.sync.dma_start(
                            xt[:], xT_src[k * P:(k + 1) * P, c * P:(c + 1) * P])
                    nc.tensor.matmul(
                        out=psf[:], lhsT=xt[:], rhs=we_tiles[k][:],
                        start=(k == 0), stop=(k == kin // P - 1),
                    )
                ftb = sb_ft.tile([P, 128], BF16, tag="ftb")
                nc.vector.tensor_copy(ftb[:], psf[:, 0:128])
                nc.sync.dma_start(ftl_d[c * P:(c + 1) * P, :], ftb[:])
                nc.vector.tensor_copy(er_stage[:, c, 0:4], psf[:, 128:132])
            nc.sync.dma_start(
                ertab_d[0:nlp, :].rearrange("(c p) h -> p c h", p=P),
                er_stage[:],
            )
            nc.sync.dma_start(ertab_d[nlp:nlp + 1, :], zer_t[:])

        def edge_phase(layer, ftf_d, ertab_d, al_rep, x_out_d, do_head=False):
            import os as _os
            estage = int(_os.environ.get("KEDGE", "9"))
            # static column offsets into the packed per-core arrays
            s_off = lo_off = hi_off = 0
            ch0 = 0
            for g in groups:
                S = g * T
                nlo, nhi = g * T_LO * P, g * T_HI * P
                # --- load idx + edge data ---
                ilo = sb.tile([P, nlo // 16], I16, tag="ilo")
                nc.sync.dma_start(ilo[:], srclo[:, lo_off // 16:(lo_off + nlo) // 16])
                ihi = sb.tile([P, nhi // 16], I16, tag="ihi")
                nc.sync.dma_start(ihi[:], srchi[:, hi_off // 16:(hi_off + nhi) // 16])
                ier = sb.tile([P, S * P // 16], I16, tag="ier")
                nc.sync.dma_start(ier[:], erix[:, s_off * 8:(s_off + S) * 8])
                dr = sb.tile([P, S], F32, tag="dr")
                nc.sync.dma_start(dr[:], dstrel[:, s_off:s_off + S])
                wc = sb.tile([P, S], F32, tag="wc")
                nc.sync.dma_start(wc[:], wcol[:, s_off:s_off + S])

                # --- gathers ---
                import os as _os2
                g_t = sb.tile([P, S, 128], BF16, tag="g_t")
                if not _os2.environ.get("KNOLO"):
                    nc.gpsimd.dma_gather(
                        out_ap=g_t[:, 0:g * T_LO, :], in_ap=ftf_d[:, :],
                        idxs_ap=ilo[:], num_idxs=nlo, num_idxs_reg=nlo, elem_size=128,
                        single_packet=False,
                    )
                if not _os2.environ.get("KNOHI"):
                    nc.gpsimd.dma_gather(
                        out_ap=g_t[:, g * T_LO:S, :], in_ap=ftf_d[hi_base:npad, :],
                        idxs_ap=ihi[:], num_idxs=nhi, num_idxs_reg=nhi, elem_size=128,
                        single_packet=False,
                    )
                er_g = sb.tile([P, S, ER_W], F32, tag="er_g")
                if not _os2.environ.get("KNOER"):
                    nc.gpsimd.dma_gather(
                        out_ap=er_g[:], in_ap=ertab_d[:, :],
                        idxs_ap=ier[:], num_idxs=S * P, num_idxs_reg=S * P, elem_size=ER_W,
                        single_packet=False,
                    )

                if estage < 2:
                    xc0 = sb.tile([P, 128], F32, tag="xck")
                    nc.vector.tensor_copy(xc0[:], g_t[:, 0, :])
                    ch = ch0
                    nc.sync.dma_start(x_out_d[ch * P:(ch + 1) * P, :], xc0[:])
                    s_off += S; lo_off += nlo; hi_off += nhi; ch0 += g
                    continue
                msg = sb.tile([P, S, 132], BF16, tag="msg")
                el = sb.tile([P, S, 4], F32, tag="el")
                au = sb.tile([P, S, 4], BF16, tag="au")
                elp = sb.tile([P, S, 128], BF16, tag="elp")

                # el: bf16 product into scratch (2x DVE mode), reduce over D
                for cg in range(g):
                    nc.vector.tensor_mul(
                        elp[:, cg * T:(cg + 1) * T, :],
                        g_t[:, cg * T:(cg + 1) * T, :],
                        al_rep[:].rearrange("p (t j) -> p t j", j=P),
                    )
                nc.vector.reduce_sum(
                    el[:],
                    elp[:].rearrange("p s (h d) -> p s h d", h=H),
                    axis=mybir.AxisListType.X,
                )
                # logit = el + er ; lrelu ; * w ; exp
                nc.vector.tensor_add(el[:], el[:], er_g[:, :, 0:4])
                nc.scalar.activation(el[:], el[:], mybir.ActivationFunctionType.Lrelu,
                                     alpha=NEG)
                nc.vector.tensor_mul(el[:], el[:],
                                     wc[:, :, None].to_broadcast([P, S, 4]))
                nc.scalar.activation(au[:], el[:], mybir.ActivationFunctionType.Exp)

                # msg = ft * au (per head), au appended as cols 128:132
                nc.vector.tensor_mul(
                    msg[:, :, 0:128].rearrange("p s (h d) -> p s h d", h=H),
                    g_t[:].rearrange("p s (h d) -> p s h d", h=H),
                    au[:, :, :, None].to_broadcast([P, S, 4, Dh]),
                )
                nc.vector.tensor_copy(msg[:, :, 128:132], au[:])

                if estage < 3:
                    xc0 = sb.tile([P, 128], F32, tag="xck")
                    nc.vector.tensor_copy(xc0[:], msg[:, 0, 0:128])
                    ch = ch0
                    nc.sync.dma_start(x_out_d[ch * P:(ch + 1) * P, :], xc0[:])
                    s_off += S; lo_off += nlo; hi_off += nhi; ch0 += g
                    continue
                # scatter: per-tile one-hot matmul accumulated per chunk
                for cg in range(g):
                    psc = ps_e.tile([P, 132], F32, space="PSUM", tag="psc")
                    oh = sb.tile([P, T, P], BF16, tag="oh")
                    slo = cg * T_LO
                    shi = g * T_LO + cg * T_HI
                    nc.vector.tensor_tensor(
                        out=oh[:, 0:T_LO, :],
                        in0=iota_t[:, None, :].to_broadcast([P, T_LO, P]),
                        in1=dr[:, slo:slo + T_LO, None].to_broadcast([P, T_LO, P]),
                        op=mybir.AluOpType.is_equal,
                    )
                    nc.vector.tensor_tensor(
                        out=oh[:, T_LO:T, :],
                        in0=iota_t[:, None, :].to_broadcast([P, T_HI, P]),
                        in1=dr[:, shi:shi + T_HI, None].to_broadcast([P, T_HI, P]),
                        op=mybir.AluOpType.is_equal,
                    )
                    for t in range(T):
                        s = (slo + t) if t < T_LO else (shi + t - T_LO)
                        nc.tensor.matmul(
                            out=psc[:], lhsT=oh[:, t, :], rhs=msg[:, s, :],
                            start=(t == 0), stop=(t == T - 1),
                        )
                    # epilogue: x = relu(u / max(s, eps))
                    s4 = sb.tile([P, 4], F32, tag="s4")
                    nc.vector.tensor_scalar_max(s4[:], psc[:, 128:132], 1e-30)
                    rinv = sb.tile([P, 4], F32, tag="rinv")
                    nc.vector.reciprocal(rinv[:], s4[:])
                    xc = sb.tile([P, 128], F32, tag="xck")
                    nc.vector.tensor_mul(
                        xc[:].rearrange("p (h d) -> p h d", h=H),
                        psc[:, 0:128].rearrange("p (h d) -> p h d", h=H),
                        rinv[:, :, None].to_broadcast([P, 4, Dh]),
                    )
                    nc.scalar.activation(xc[:], xc[:], mybir.ActivationFunctionType.Relu)
                    ch = ch0 + cg
                    nc.sync.dma_start(x_out_d[ch * P:(ch + 1) * P, :], xc[:])
                    if do_head:
                        # fused output head: out = x1 @ Wout_top + x2 @ Wout_bot + bout
                        xh_ps = ps_ft.tile([P, P], F32, space="PSUM", tag="xt_ps")
                        nc.tensor.transpose(xh_ps[:], xc[:], ident_t[:])
                        xht = sb_ft.tile([P, P], F32, tag="xht")
                        nc.vector.tensor_copy(xht[:], xh_ps[:])
                        pso = ps_ft.tile([P, 132], F32, space="PSUM", tag="psf",
                                         name=f"pso_{ch}")[:, 0:128]
                        nc.tensor.matmul(out=pso[:], lhsT=xht[:], rhs=wout_t[1][:],
                                         start=True, stop=False)
                        xh1 = sb_ft.tile([P, P], F32, tag="xh1")
                        nc.sync.dma_start(xh1[:], x1l[ch * P:(ch + 1) * P, :])
                        xh1_ps = ps_ft.tile([P, P], F32, space="PSUM", tag="xt_ps")
                        nc.tensor.transpose(xh1_ps[:], xh1[:], ident_t[:])
                        xh1t = sb_ft.tile([P, P], F32, tag="xht")
                        nc.vector.tensor_copy(xh1t[:], xh1_ps[:])
                        nc.tensor.matmul(out=pso[:], lhsT=xh1t[:], rhs=wout_t[0][:],
                                         start=False, stop=False)
                        nc.tensor.matmul(out=pso[:], lhsT=ones_t[:], rhs=bout_t[:],
                                         start=False, stop=True)
                        oc = sb_ft.tile([P, 128], F32, tag="oc")
                        nc.vector.tensor_copy(oc[:], pso[:])
                        nc.sync.dma_start(out_local[ch * P:(ch + 1) * P, :], oc[:])
                s_off += S
                lo_off += nlo
                hi_off += nhi
                ch0 += g

        # ================= layer 1 =================
        import os
        bisect = os.environ.get("KBISECT", "")
        KREP = int(os.environ.get("KREP", "1"))
        for _rep in range(KREP):
            ft_phase(1, x0, kin1, w1e_t, ftl[0], ertab[0])
            if bisect == "ft":
                nc.gpsimd.dma_start(out_local[:, :], ftl[0][:, :])
            if not bisect or bisect in ("ag", "l1", "l2"):
                if os.environ.get("KNOAG"):
                    nc.gpsimd.dma_start(ftf[0][0:nlp, :], ftl[0][:, :])
                else:
                    nc.gpsimd.collective_compute(
                        "AllGather", mybir.AluOpType.bypass, replica_groups=rg,
                        ins=[ftl[0][:]], outs=[ftf[0][:]],
                    )
                if bisect == "ag":
                    nc.gpsimd.dma_start(out_local[:, :], ftf[0][0:nlp, :])
            if not bisect or bisect in ("l1", "l2"):
                edge_phase(1, ftf[0], ertab[0], al_t[0], x1l)
                if bisect == "l1":
                    nc.gpsimd.dma_start(out_local[:, :], x1l[:, :])

            # ================= layer 2 =================
            if not bisect or bisect == "l2":
                ft_phase(2, x1l, 128, [w2e_t], ftl[1], ertab[1])
                if os.environ.get("KNOAG"):
                    nc.gpsimd.dma_start(ftf[1][0:nlp, :], ftl[1][:, :])
                else:
                    nc.gpsimd.collective_compute(
                        "AllGather", mybir.AluOpType.bypass, replica_groups=rg,
                        ins=[ftl[1][:]], outs=[ftf[1][:]],
                    )
                edge_phase(2, ftf[1], ertab[1], al_t[1], x2l, do_head=(not bisect))
                if bisect == "l2":
                    nc.gpsimd.dma_start(out_local[:, :], x2l[:, :])

            # ================= output head (fused into layer-2 epilogue) =======
            for c in range(0):
                pso = ps_ft.tile([P, 132], F32, space="PSUM", tag="psf", name=f"pso{c}")[:, 0:128]
                for k, xsrc in ((0, x1l), (1, x2l)):
                    xh = sb_ft.tile([P, P], F32, tag="xh")
                    nc.sync.dma_start(xh[:], xsrc[c * P:(c + 1) * P, :])
                    xh_ps = ps_ft.tile([P, P], F32, space="PSUM", tag="xt_ps")
                    nc.tensor.transpose(xh_ps[:], xh[:], ident_t[:])
                    xht = sb_ft.tile([P, P], F32, tag="xht")
                    nc.vector.tensor_copy(xht[:], xh_ps[:])
                    nc.tensor.matmul(out=pso[:], lhsT=xht[:], rhs=wout_t[k][:],
                                     start=(k == 0), stop=False)
                nc.tensor.matmul(out=pso[:], lhsT=ones_t[:], rhs=bout_t[:],
                                 start=False, stop=True)
                oc = sb_ft.tile([P, 128], F32, tag="oc")
                nc.vector.tensor_copy(oc[:], pso[:])
                nc.sync.dma_start(out_local[c * P:(c + 1) * P, :], oc[:])

    nc.compile()
    return nc


# ----------------------------------------------------------------------------
# public entry point
# ----------------------------------------------------------------------------
def _run(features, src, dst, w, W1, al1, ar1, W2, al2, ar2, Wout, bout,
         trace=False):
    n_nodes = features.shape[0]
    kin1 = features.shape[1]
    nloc = math.ceil(n_nodes / NCORE)            # 6250
    nlp = math.ceil(nloc / P) * P                # 6272
    import os as _osG
    G = int(_osG.environ.get("KG", "4"))

    features = np.asarray(features, np.float32)
    src = np.asarray(src, np.int32)
    dst = np.asarray(dst, np.int32)
    w = np.asarray(w, np.float32)

    plan = _edge_plan(src, dst, w, n_nodes, nloc, nlp, G)
    T = plan["T_LO"] + plan["T_HI"]

    def ext(W, ar):
        wr = (np.asarray(W, np.float32).reshape(W.shape[0], H, Dh)
              * np.asarray(ar, np.float32)[None]).sum(-1)
        return np.concatenate([np.asarray(W, np.float32), wr], axis=1)

    w1e = ext(W1, ar1)
    w2e = ext(W2, ar2)
    al1f = np.asarray(al1, np.float32).reshape(-1)
    al2f = np.asarray(al2, np.float32).reshape(-1)
    import ml_dtypes
    al1rep = np.tile(al1f[None, :], (P, T)).astype(ml_dtypes.bfloat16)
    al2rep = np.tile(al2f[None, :], (P, T)).astype(ml_dtypes.bfloat16)
    iota_np = np.tile(np.arange(P, dtype=np.float32)[None, :], (P, 1))

    common = dict(
        w1e=w1e, w2e=w2e, wout=np.asarray(Wout, np.float32),
        boutr=np.asarray(bout, np.float32).reshape(1, 128),
        al1r=al1rep, al2r=al2rep, iota=iota_np,
        onesr=np.ones((1, 128), np.float32),
    )
    in_maps = []
    for c in range(NCORE):
        lo = c * nloc
        hi = min((c + 1) * nloc, n_nodes)
        x0 = np.zeros((nlp, kin1), np.float32)
        x0[:hi - lo] = features[lo:hi]
        pc = plan["cores"][c]
        in_maps.append(dict(
            x0=x0, srclo=pc["srclo_w"], srchi=pc["srchi_w"], erix=pc["erix_w"],
            dstrel=pc["dstrel"], wcol=pc["wcol"], **common,
        ))

    prog = _build_program(n_nodes, nloc, nlp, kin1, plan)
    global _LAST_PROG
    _LAST_PROG = (prog, in_maps)
    res = run_bass_kernel_spmd(prog, in_maps, list(range(NCORE)), trace=trace)

    outs = []
    for c in range(NCORE):
        lo = c * nloc
        hi = min((c + 1) * nloc, n_nodes)
        outs.append(res.results[c]["out_local"][:hi - lo])
    full = np.concatenate(outs, axis=0)
    return full, res


def kernel(features, src, dst, w, W1, al1, ar1, W2, al2, ar2, Wout, bout):
    out, _ = _run(features, src, dst, w, W1, al1, ar1, W2, al2, ar2, Wout, bout)
    return out

